# revision 1
# baseline (speedup 1.0000x reference)
"""Trainium2 Bass kernel for the counting-criterion loss.

Computes, for output/density_map of shape [32, 1, 512, 512] and bboxes [32, 3, 4]:
  dmap_loss  = sum((output - density_map)^2) / num_objects
  count_loss = mean_b((sum(output_b) - sum(density_map_b))^2)
  min_count  = sum_boxes(relu(1 - box_sum))   with box sums over [y1:y2, x1:x2)

Strategy: data-parallel over the batch — core i handles images [4i, 4i+4).
On each core, per image:
  - one DVE scalar_tensor_tensor gives diff = o - d plus per-partition sum(diff)
  - one ACT Square activation with accum_out gives per-partition sum(diff^2)
  - box sums via PE: for each x-chunk, O_chunk^T (stationary [128,128]) @
    rowmask (moving [128,3]) accumulated over the 4 y-chunks -> psum[x, (c,j)];
    multiply by the column mask on DVE, then a ones-vector matmul reduces over
    the x partitions.
Final tiny reductions (cross-partition sums, relu, squares, weights) run on
the host from each core's [128,4]+[128,4]+[1,48] partial outputs.
"""

import numpy as np
from contextlib import ExitStack

import concourse.bass as bass
import concourse.mybir as mybir
import concourse.tile as tile
from concourse import bacc
from concourse.bass_utils import run_bass_kernel_spmd

N_CORES = 8
B, H, W = 32, 512, 512
NIMG = B // N_CORES  # images per core
P = 128              # SBUF partitions
NCH = H // P         # row chunks per image (and col chunks: W//P)
NB = 3               # boxes per image
F32 = mybir.dt.float32

_PROG = None


def _build_program():
    nc = bacc.Bacc(
        "TRN2",
        target_bir_lowering=False,
        debug=False,
        num_devices=N_CORES,
    )
    o_d = nc.dram_tensor("o", [NIMG, H, W], F32, kind="ExternalInput").ap()
    d_d = nc.dram_tensor("d", [NIMG, H, W], F32, kind="ExternalInput").ap()
    # packed masks per image: cols 0:NCH*NB row mask [y%128, (y//128, j)],
    # cols NCH*NB:2*NCH*NB col mask [x%128, (x//128, j)]
    msk_d = nc.dram_tensor(
        "msk", [NIMG, P, 2 * NCH * NB], F32, kind="ExternalInput"
    ).ap()
    # columns: img0..img2 as 2 halves each, then img3 as 3 quarters + 2
    # eighths; first NCOL are sum(diff) partials, next NCOL are sum(diff^2)
    # partials, then 48 box partials (row 0 only: img-major (img, cx, j))
    NCOL = 2 * (NIMG - 1) + NCH + 1
    NBOXCOL = NIMG * NCH * NB
    acc_d = nc.dram_tensor(
        "acc", [P, 2 * NCOL + NBOXCOL], F32, kind="ExternalOutput"
    ).ap()

    # DRAM views: image rows split as y = c*128 + p  ->  [img, p, c, x]
    o_r = o_d.rearrange("n (c p) x -> n p c x", p=P)
    d_r = d_d.rearrange("n (c p) x -> n p c x", p=P)

    with tile.TileContext(nc) as tc, ExitStack() as ctx:
        io_pool = ctx.enter_context(tc.tile_pool(name="io", bufs=2))
        qio_pool = ctx.enter_context(tc.tile_pool(name="qio", bufs=1))
        mask_pool = ctx.enter_context(tc.tile_pool(name="mask", bufs=2))
        work_pool = ctx.enter_context(tc.tile_pool(name="work", bufs=2))
        psum_pool = ctx.enter_context(tc.tile_pool(name="psum", bufs=2, space="PSUM"))
        acc_pool = ctx.enter_context(tc.tile_pool(name="acc", bufs=1))

        acc = acc_pool.tile([P, 2 * NCOL + NBOXCOL], F32)
        nc.vector.memset(acc[:], 0.0)
        ones_t = acc_pool.tile([P, 1], F32)
        nc.vector.memset(ones_t[:], 1.0)

        def box_work(img, o_chunks, msk_t):
            """o_chunks: list of (tile, free-index) giving [128, 512] y-chunk APs."""
            ps = psum_pool.tile([P, NCH * NB], F32, tag="ps")
            for cx in range(NCH):
                for cy in range(NCH):
                    t, idx = o_chunks[cy]
                    nc.tensor.matmul(
                        ps[:, cx * NB : (cx + 1) * NB],
                        lhsT=t[:, idx, cx * P : (cx + 1) * P],
                        rhs=msk_t[:, cy * NB : (cy + 1) * NB],
                        start=(cy == 0),
                        stop=(cy == NCH - 1),
                    )
            masked_t = work_pool.tile([P, NCH * NB], F32, tag="masked")
            nc.vector.tensor_tensor(
                out=masked_t[:],
                in0=ps[:],
                in1=msk_t[:, NCH * NB : 2 * NCH * NB],
                op=mybir.AluOpType.mult,
            )
            ps2 = psum_pool.tile([1, NCH * NB], F32, tag="ps2")
            nc.tensor.matmul(
                ps2[:], lhsT=ones_t[:], rhs=masked_t[:], start=True, stop=True
            )
            col0 = 2 * NCOL + img * NCH * NB
            nc.vector.tensor_copy(acc[0:1, col0 : col0 + NCH * NB], ps2[:])

        def diff_work(o_ap, d_ap, col, square_on_dve=False, tag=""):
            """stt diff + square over one chunk, accumulating into column col.

            The square runs on ACT by default (hides under DMA); for the tail
            chunks it runs on DVE so the critical chain stays on one engine.
            """
            diff_t = work_pool.tile(
                list(o_ap.shape), F32, tag="diff" + tag, bufs=5 if tag else None
            )
            nc.vector.scalar_tensor_tensor(
                out=diff_t[:],
                in0=o_ap,
                scalar=0.0,
                in1=d_ap,
                op0=mybir.AluOpType.bypass,
                op1=mybir.AluOpType.subtract,
                accum_out=acc[:, col : col + 1],
            )
            sq_t = work_pool.tile(
                list(o_ap.shape), F32, tag="sq" + tag, bufs=5 if tag else None
            )
            if square_on_dve:
                nc.vector.scalar_tensor_tensor(
                    out=sq_t[:],
                    in0=diff_t[:],
                    scalar=0.0,
                    in1=diff_t[:],
                    op0=mybir.AluOpType.bypass,
                    op1=mybir.AluOpType.mult,
                    accum_out=acc[:, NCOL + col : NCOL + col + 1],
                )
            else:
                nc.scalar.activation(
                    sq_t[:],
                    diff_t[:],
                    mybir.ActivationFunctionType.Square,
                    accum_out=acc[:, NCOL + col : NCOL + col + 1],
                )

        msk_all = mask_pool.tile([P, NIMG, 2 * NCH * NB], F32)

        # images 0..NIMG-2: half-image pipeline (keeps DVE/ACT streaming
        # steadily behind the DMA instead of big 2.2us blocks)
        HC = NCH // 2
        for img in range(NIMG - 1):
            halves = []
            for h in range(2):
                o_t = io_pool.tile([P, HC, W], F32, tag=f"o{h}")
                nc.sync.dma_start(o_t[:], o_r[img, :, h * HC : (h + 1) * HC])
                d_t = io_pool.tile([P, HC, W], F32, tag=f"d{h}")
                nc.sync.dma_start(d_t[:], d_r[img, :, h * HC : (h + 1) * HC])
                if img == 0 and h == 0:
                    # all masks in one small DMA, tucked behind the first pair
                    nc.sync.dma_start(
                        msk_all[:], msk_d.rearrange("n p m -> p n m")
                    )
                diff_work(o_t[:], d_t[:], 2 * img + h)
                halves.append(o_t)
            box_work(
                img,
                [(halves[c // HC], c % HC) for c in range(NCH)],
                msk_all[:, img],
            )

        # last image: quarter-chunks with interleaved o/d DMAs (last quarter as
        # two eighths) so the post-DMA tail is only an eighth-image chain
        img = NIMG - 1
        oq_tiles, chunks = [], []
        for c in range(NCH):
            if c < NCH - 1:
                oq = qio_pool.tile([P, 1, W], F32, tag=f"oq{c}")
                nc.sync.dma_start(oq[:], o_r[img, :, c : c + 1])
                dq = qio_pool.tile([P, 1, W], F32, tag=f"dq{c}")
                nc.sync.dma_start(dq[:], d_r[img, :, c : c + 1])
                oq_tiles.append((oq, 0))
                chunks.append((oq[:], dq[:]))
            else:
                # final quarter as two eighth-image pieces
                oq = qio_pool.tile([P, 1, W], F32, tag=f"oq{c}")
                dq = qio_pool.tile([P, 1, W], F32, tag=f"dq{c}")
                for h in range(2):
                    hs = slice(h * (W // 2), (h + 1) * (W // 2))
                    nc.sync.dma_start(oq[:, 0, hs], o_r[img, :, c, hs])
                    nc.sync.dma_start(dq[:, 0, hs], d_r[img, :, c, hs])
                    chunks.append((oq[:, 0, hs], dq[:, 0, hs]))
                oq_tiles.append((oq, 0))
        for i, (o_ap, d_ap) in enumerate(chunks):
            # the very last chunk squares on DVE: keeps the critical chain on
            # one engine with no cross-engine semaphore hop
            diff_work(
                o_ap,
                d_ap,
                2 * (NIMG - 1) + i,
                square_on_dve=(i == len(chunks) - 1),
                tag="q",
            )
        box_work(img, oq_tiles, msk_all[:, img])

        nc.sync.dma_start(acc_d, acc[:])

    nc.compile()
    return nc


def _get_program():
    global _PROG
    if _PROG is None:
        _PROG = _build_program()
    return _PROG


def _prep_inputs(output, density_map, bboxes):
    o = np.ascontiguousarray(np.asarray(output, dtype=np.float32).reshape(B, H, W))
    dm = np.ascontiguousarray(
        np.asarray(density_map, dtype=np.float32).reshape(B, H, W)
    )
    bb = np.clip(np.asarray(bboxes).astype(np.int64), 0, W).astype(np.int32)
    x1, y1, x2, y2 = bb[..., 0], bb[..., 1], bb[..., 2], bb[..., 3]
    x2 = np.maximum(x2, x1)
    y2 = np.maximum(y2, y1)

    ar = np.arange(H, dtype=np.int32)
    # rm[b, y, j] = 1 if y1 <= y < y2, laid out as [b, y%128, (y//128, j)]
    rm = (
        (ar[None, :, None] >= y1[:, None, :]) & (ar[None, :, None] < y2[:, None, :])
    ).astype(np.float32)
    rm = rm.reshape(B, NCH, P, NB).transpose(0, 2, 1, 3).reshape(B, P, NCH * NB)
    # cm[b, j, x] = 1 if x1 <= x < x2, laid out as [b, x%128, (x//128, j)]
    cm = (
        (ar[None, None, :] >= x1[:, :, None]) & (ar[None, None, :] < x2[:, :, None])
    ).astype(np.float32)
    cm = cm.reshape(B, NB, NCH, P).transpose(0, 3, 2, 1).reshape(B, P, NCH * NB)
    msk = np.ascontiguousarray(np.concatenate([rm, cm], axis=2))  # [B, P, 24]
    return o, dm, msk


def kernel(output, density_map, bboxes, num_objects):
    o, dm, msk = _prep_inputs(output, density_map, bboxes)

    nc = _get_program()
    in_maps = [
        {
            "o": o[i * NIMG : (i + 1) * NIMG],
            "d": dm[i * NIMG : (i + 1) * NIMG],
            "msk": msk[i * NIMG : (i + 1) * NIMG],
        }
        for i in range(N_CORES)
    ]
    res = run_bass_kernel_spmd(nc, in_maps, core_ids=list(range(N_CORES)))

    NCOL = 2 * (NIMG - 1) + NCH + 1

    def _per_img(cols):
        # columns: img0..img2 as 2 halves each, img3 as its remaining chunks
        firsts = [cols[2 * i] + cols[2 * i + 1] for i in range(NIMG - 1)]
        return np.array(firsts + [cols[2 * (NIMG - 1) :].sum()])

    per_img_d = np.concatenate(
        [
            _per_img(r["acc"][:, :NCOL].sum(axis=0, dtype=np.float64))
            for r in res.results
        ]
    )  # [B] sum(o - d) per image
    sq_total = float(
        sum(r["acc"][:, NCOL : 2 * NCOL].sum(dtype=np.float64) for r in res.results)
    )  # sum((o - d)^2)
    # acc[0, 2*NCOL + (img, cx, j)] -> sum over cx -> [NIMG, NB], image-major
    box_sums = np.concatenate(
        [
            r["acc"][0, 2 * NCOL :]
            .reshape(NIMG, NCH, NB)
            .sum(axis=1, dtype=np.float64)
            .reshape(-1)
            for r in res.results
        ]
    )  # [B*NB]

    dmap_loss = sq_total / float(num_objects)
    count_loss = float(np.mean(per_img_d**2))
    min_count = float(np.maximum(0.0, 1.0 - box_sums).sum())
    return np.array([dmap_loss, count_loss, min_count], dtype=np.float32)

